# revision 19
# baseline (speedup 1.0000x reference)
"""Causal self-attention (RoPE + qk-RMS-norm) Trainium2 kernel.

Sharding: 8 cores = 2 batches x 4 head-groups (tensor-parallel over heads,
data-parallel over batch). Each core computes its head-group's attention and
a row-parallel partial of the output projection; the host sums the 4
per-group partials per batch (the all-reduce of row-parallel sharding).

All matmul operands are bf16 (PSUM accumulation stays fp32); norm scalars
and softmax denominators stay fp32. RoPE is computed without the tensor
engine: a SBUF->SBUF DMA swaps the two halves of the head dim, then
y = q*[cos;cos] + swap(q)*[sin;-sin] on the vector/gpsimd engines. The
RMS-norm sum-of-squares is taken from the pre-rope values (rotation
preserves norms), so the norm chain runs in parallel with the rope chain.
Attention matmuls (S.T, PV, colsum) and the exp are narrowed to the causal
span inside diagonal 512-query supertiles. Tokens are processed in two
causal passes; each pass's last-head attention is carried into the next
pass (or the epilogue) so projection matmuls hide its exp latency.
"""

import functools

import numpy as np

B, T, C, H, D = 2, 2048, 1280, 10, 128
EPS = 1e-5
NHL = 3  # head slots per core (padded)
N_CORES = 8
NHALF = 2  # causal passes over T
NWARM = 48
# per-batch head groups (4th group padded with zero heads)
GROUPS = [[0, 1, 2], [3, 4, 5], [6, 7, 8], [9]]


def _emit(nc, tile, mybir, T, C, D, NHL, eps):
    F32 = mybir.dt.float32
    F32R = mybir.dt.float32r
    BF16 = mybir.dt.bfloat16
    ActF = mybir.ActivationFunctionType
    CCH = C // 128  # contraction chunks
    TBN = T // 128  # 128-token blocks
    T2 = T // NHALF  # tokens per pass
    TB2 = T2 // 128
    Q42 = T2 // 512  # q supertiles per pass
    HD = NHL * D
    couts = []
    off = 0
    while off < C:
        w = min(512, C - off)
        couts.append((off, w))
        off += w

    xt = nc.dram_tensor("xt", [C, T], BF16, kind="ExternalInput")
    wqt = nc.dram_tensor("wqt", [C, HD], BF16, kind="ExternalInput")
    wkt = nc.dram_tensor("wkt", [C, HD], BF16, kind="ExternalInput")
    wvt = nc.dram_tensor("wvt", [C, HD], BF16, kind="ExternalInput")
    wpt = nc.dram_tensor("wpt", [HD, C], BF16, kind="ExternalInput")
    csd = nc.dram_tensor("csd", [D, T], BF16, kind="ExternalInput")  # [cos;cos]
    ssd = nc.dram_tensor("ssd", [D, T], BF16, kind="ExternalInput")  # [sin;-sin]
    out = nc.dram_tensor("out", [T, C], BF16, kind="ExternalOutput")

    from contextlib import ExitStack

    with ExitStack() as ctx:
        ctx.enter_context(nc.allow_low_precision(reason="bf16 matmul pipeline"))
        tc = ctx.enter_context(tile.TileContext(nc))
        pool = lambda n, b, **kw: ctx.enter_context(tc.tile_pool(name=n, bufs=b, **kw))
        drp = pool("dr", 2, space="DRAM")
        per = pool("persist", 1)
        wvp = pool("wv", 1)
        wqkp = pool("wqk", 1)
        wptp = pool("wpt", 1)
        xtp = pool("xt", 1)
        qtp = pool("qt", 2)
        qxp = pool("qx", 1)
        ytp = pool("yt", 2)
        tmp = pool("tmp", 4)
        sqp = pool("sqp", 1)
        ptp = pool("ptp", 4)
        rows = pool("rows", 3)
        oev = pool("oev", 2)
        psmm = pool("psmm", 2, space="PSUM")
        psacc = pool("psacc", 2, space="PSUM")
        psrow = pool("psrow", 2, space="PSUM")

        # ---- constants ----
        ones_f = per.tile([128, 128], F32, tag="onf")
        nc.vector.memset(ones_f[:], 1.0)
        ones_col = per.tile([128, 1], BF16, tag="onc")
        nc.scalar.copy(ones_col[:], ones_f[:, 0:1])
        ones_row = per.tile([1, 128], BF16, tag="onr")
        nc.scalar.copy(ones_row[:], ones_f[0:1, :])
        beps_col = per.tile([128, 1], F32, tag="bepsc")
        nc.vector.memset(beps_col[:], float(eps))
        # 0/1 mask: keep tq >= tk in [tk, tq] layout (upper incl diag)
        tri01 = per.tile([128, 128], BF16, tag="tri")
        nc.vector.memset(tri01[:], 1.0)
        nc.gpsimd.affine_select(
            out=tri01[:], in_=tri01[:],
            compare_op=mybir.AluOpType.is_ge,
            fill=0.0, base=0,
            pattern=[[1, 128]], channel_multiplier=-1,
        )

        # PE warm-up: dummy accumulating matmuls during the initial DMA ramp
        warm = nc.dram_tensor("warm", [1, 512], F32, kind="ExternalOutput")
        wrhs = per.tile([128, 512], BF16, tag="wrhs")
        for i in range(4):
            nc.scalar.copy(wrhs[:, i * 128 : (i + 1) * 128], ones_f[:])
        wps = psrow.tile([1, 512], F32, tag="row", name="warmps")
        for i in range(NWARM):
            nc.tensor.matmul(
                wps[:], ones_col[:], wrhs[:], start=(i == 0), stop=(i == NWARM - 1)
            )
        wsb = rows.tile([1, 512], F32, tag="rw", name="warmsb")
        nc.vector.tensor_copy(wsb[:], wps[:])
        nc.sync.dma_start(warm[:], wsb[:])

        # V for all heads/all tokens: [tk-part, tb, h, d]
        v_t = per.tile([128, TBN, NHL, D], BF16, tag="v")
        # K.T per head, all tokens
        ktr = [per.tile([128, T], BF16, tag=f"ktr{h}", name=f"ktr{h}")
               for h in range(NHL)]
        rk_cols = [per.tile([128, TBN], F32, tag=f"rkc{h}", name=f"rkc{h}")
                   for h in range(NHL)]

        # ---- all weights resident, loaded upfront on the sync queue ----
        # x.T for both passes: pass 0 on sync (needed first), pass 1 on gpsimd
        xc_all = []
        for hf in range(NHALF):
            xs = []
            for c in range(CCH):
                t = xtp.tile([128, T2], BF16, tag=f"x{c}p{hf}")
                eng = nc.sync if hf == 0 else nc.gpsimd
                eng.dma_start(
                    t[:], xt[c * 128 : (c + 1) * 128, hf * T2 : (hf + 1) * T2]
                )
                xs.append(t)
            xc_all.append(xs)
        wqall = []
        wkall = []
        for c in range(CCH):
            tq = wqkp.tile([128, HD], BF16, tag=f"wq{c}")
            nc.sync.dma_start(tq[:], wqt[c * 128 : (c + 1) * 128, :])
            wqall.append(tq)
            tk = wqkp.tile([128, HD], BF16, tag=f"wk{c}")
            nc.sync.dma_start(tk[:], wkt[c * 128 : (c + 1) * 128, :])
            wkall.append(tk)
        wv = []
        for c in range(CCH):
            t = wvp.tile([128, HD], BF16, tag=f"wv{c}")
            nc.sync.dma_start(t[:], wvt[c * 128 : (c + 1) * 128, :])
            wv.append(t)
        # cos/sin stacks for both passes
        cs_t = []
        ss_t = []
        for hf in range(NHALF):
            tc_ = qtp.tile([D, T2], BF16, tag=f"cs{hf}", bufs=1)
            ts_ = qtp.tile([D, T2], BF16, tag=f"ss{hf}", bufs=1)
            eng = nc.sync if hf == 0 else nc.gpsimd
            eng.dma_start(tc_[:], csd[:, hf * T2 : (hf + 1) * T2])
            eng.dma_start(ts_[:], ssd[:, hf * T2 : (hf + 1) * T2])
            cs_t.append(tc_)
            ss_t.append(ts_)
        # output-projection weights (resident)
        wp = {}
        for hh in range(NHL):
            for ci, (co, cw) in enumerate(couts):
                t = wptp.tile([128, cw], BF16, tag=f"wp{hh}_{ci}")
                nc.sync.dma_start(
                    t[:], wpt[hh * 128 : (hh + 1) * 128, co : co + cw]
                )
                wp[(hh, ci)] = t

        def emit_attention(hf, h, qtn, ytn):
            """Attention for head h over this pass's q supertiles.
            kb-outer; st/exp run LA kb steps ahead of PV/colsum. Matmuls and
            the exp are narrowed to the causal span inside diagonal
            supertiles."""
            gq4s = [hf * Q42 + q4 for q4 in range(Q42)]
            yts = [psacc.tile([128, 512], F32, tag="acc", name=f"yt{q4}")
                   for q4 in range(Q42)]
            csums = [psrow.tile([1, 512], F32, tag="row", name=f"cs{q4}")
                     for q4 in range(Q42)]
            kbmax = 4 * (gq4s[-1] + 1)
            LA = 3  # st/exp run this many kb steps ahead of PV/colsum
            pts = {}  # kb -> pt tile awaiting PV/colsum

            def span(q4, kb):
                # valid columns of supertile q4 at key block kb (None if none)
                j = kb - 4 * gq4s[q4]
                if j > 3:
                    return None
                return max(j, 0) * 128

            for kb in range(kbmax + LA):
                if kb < kbmax:
                    active = [q4 for q4 in range(Q42) if kb <= 4 * gq4s[q4] + 3]
                    st = psmm.tile([128, Q42 * 512], F32, tag="mm", name="st")
                    for q4 in active:
                        a0 = q4 * 512 + span(q4, kb)
                        hi4 = (q4 + 1) * 512
                        nc.tensor.matmul(
                            st[:, a0:hi4],
                            ktr[h][:, kb * 128 : (kb + 1) * 128],
                            qtn[:, a0:hi4],
                            start=True, stop=True,
                        )
                    pt = ptp.tile([128, Q42 * 512], BF16, tag="pt")
                    lo = active[0] * 512 + span(active[0], kb)
                    hi = (active[-1] + 1) * 512
                    nc.scalar.activation(
                        pt[:, lo:hi], st[:, lo:hi], ActF.Exp,
                        scale=rk_cols[h][:, kb : kb + 1],
                    )
                    for q4 in active:
                        j = kb - 4 * gq4s[q4]
                        if 0 <= j <= 3:
                            dg = slice(q4 * 512 + j * 128, q4 * 512 + (j + 1) * 128)
                            nc.vector.tensor_mul(pt[:, dg], pt[:, dg], tri01[:])
                    pts[kb] = pt
                if kb >= LA:
                    pkb = kb - LA
                    pt = pts.pop(pkb)
                    for q4 in range(Q42):
                        gq4 = gq4s[q4]
                        last_kb = 4 * gq4 + 3
                        if pkb > last_kb:
                            continue
                        a0 = span(q4, pkb)
                        nc.tensor.matmul(
                            yts[q4][:, a0:512],
                            v_t[:, pkb, h, :],
                            pt[:, q4 * 512 + a0 : (q4 + 1) * 512],
                            start=(pkb == 0), stop=(pkb == last_kb),
                            skip_group_check=True,
                        )
                        nc.tensor.matmul(
                            csums[q4][:, a0:512],
                            ones_col[:],
                            pt[:, q4 * 512 + a0 : (q4 + 1) * 512],
                            start=(pkb == 0), stop=(pkb == last_kb),
                            skip_group_check=True,
                        )
            # 1/colsum rows: recip straight from PSUM, then a rounding copy
            # into F32R (walrus requires fp32r matmul inputs to be rounded)
            rrrs = []
            for q4 in range(Q42):
                rr = rows.tile([1, 512], F32, tag="rw", name="rr")
                nc.vector.reciprocal_approx_fast(rr[:], csums[q4][:])
                rrr = rows.tile([1, 512], BF16, tag="rr", name="rrr")
                nc.vector.tensor_copy(rrr[:], rr[:])
                rrrs.append(rrr)

            def normalize(h=h, ytn=ytn, yts=yts, rrrs=rrrs):
                for q4 in range(Q42):
                    lsl = slice(q4 * 512, (q4 + 1) * 512)
                    bc = psmm.tile([128, 512], F32, tag="mm", name="bc")
                    nc.tensor.matmul(
                        bc[:], ones_row[:], rrrs[q4][:], start=True, stop=True
                    )
                    # DVE can read only one PSUM operand: evict bc via ACT
                    bcb = tmp.tile([128, 512], F32, tag="bcb")
                    nc.scalar.copy(bcb[:], bc[:])
                    nc.vector.tensor_mul(ytn[:, h, lsl], yts[q4][:], bcb[:])

            return normalize

        pending = None  # deferred attention emitter for the previous head
        post_attn = None  # deferred out-projection for the previous pass

        for hf in range(NHALF):
            toff = hf * T2
            xc = xc_all[hf]
            cst = cs_t[hf]
            sst = ss_t[hf]

            def emit_vproj(hf=hf, xc=xc):
                # V projection for this pass, all heads batched. Emitted
                # after head 0's Q/K projections so its PE matmuls cover
                # head 0's rope/norm chain latency.
                for tb in range(TB2):
                    gtb = hf * TB2 + tb
                    vp = psmm.tile([128, HD], F32, tag="mm", name="vp")
                    for c in range(CCH):
                        nc.tensor.matmul(
                            vp[:],
                            xc[c][:, tb * 128 : (tb + 1) * 128],
                            wv[c][:],
                            start=(c == 0), stop=(c == CCH - 1),
                        )
                    nc.vector.tensor_copy(v_t[:, gtb, :, :], vp[:])

            # Y.T for this pass (all heads)
            ytn = ytp.tile([128, NHL, T2], BF16, tag="ytn")

            for h in range(NHL):
                # ---- Q/K projections into PSUM; evict + swap + squares ----
                qunits = {}  # (isq, q4) -> (qx, qsw, sq)
                for isq, wt in enumerate((wqall, wkall)):
                    qps = psmm.tile([128, Q42 * 512], F32, tag="mm", name="qps")
                    for c in range(CCH):
                        for q4 in range(Q42):
                            nc.tensor.matmul(
                                qps[:, q4 * 512 : (q4 + 1) * 512],
                                wt[c][:, h * D : (h + 1) * D],
                                xc[c][:, q4 * 512 : (q4 + 1) * 512],
                                start=(c == 0), stop=(c == CCH - 1),
                            )
                    for q4 in range(Q42):
                        qx = qxp.tile([128, 512], BF16, tag=f"qx{isq}{q4}")
                        nc.vector.tensor_copy(qx[:], qps[:, q4 * 512 : (q4 + 1) * 512])
                        qsw = qxp.tile([128, 512], BF16, tag=f"qw{isq}{q4}")
                        nc.sync.dma_start(qsw[0:64, :], qx[64:128, :])
                        nc.sync.dma_start(qsw[64:128, :], qx[0:64, :])
                        qunits[(isq, q4)] = (qx, qsw)

                # ---- previous head's attention (dense PE block) ----
                if pending is not None:
                    norm_prev = pending()
                    pending = None
                else:
                    norm_prev = None

                qtn = qtp.tile([128, T2], BF16, tag="qtn")

                if norm_prev is not None:
                    norm_prev()
                if post_attn is not None:
                    post_attn()
                    post_attn = None
                if h == 0:
                    emit_vproj()

                # ---- rope combine (no PE): dst = qx*CS + swap(qx)*SS ----
                # (cos/sin here are arbitrary tensors, so rope does NOT
                # preserve norms -- squares must come from post-rope values)
                combs = {}
                for isq in range(2):
                    for q4 in range(Q42):
                        qx, qsw = qunits[(isq, q4)]
                        lsl4 = slice(q4 * 512, (q4 + 1) * 512)
                        qc = tmp.tile([128, 512], BF16, tag="t1")
                        nc.vector.tensor_mul(qc[:], qx[:], cst[:, lsl4])
                        t2 = tmp.tile([128, 512], BF16, tag="t2")
                        nc.gpsimd.tensor_mul(t2[:], qsw[:], sst[:, lsl4])
                        combs[(isq, q4)] = (qc, t2)
                sqs = {}
                for isq, (dst, doff) in enumerate(((qtn, 0), (ktr[h], toff))):
                    for q4 in range(Q42):
                        qc, t2 = combs[(isq, q4)]
                        dsl = slice(doff + q4 * 512, doff + (q4 + 1) * 512)
                        nc.vector.tensor_add(dst[:, dsl], qc[:], t2[:])
                        sq = sqp.tile([128, 512], BF16, tag=f"sq{isq}{q4}")
                        nc.vector.tensor_mul(sq[:], dst[:, dsl], dst[:, dsl])
                        sqs[(isq, q4)] = sq

                # ---- norms from post-rope squares ----
                # rsqrt via exp(-0.5*ln(v)): ln/exp/copy live in ONE act
                # table set, so the scalar engine never reloads tables
                # (Sqrt shares no table with Exp and each reload is 1.3us).
                # k: rk row, then one strided DMA transpose via DRAM bounce
                rkrow = rows.tile([1, T2], F32, tag="rkrow", bufs=2)
                for q4 in range(Q42):
                    lsl = slice(q4 * 512, (q4 + 1) * 512)
                    ssk = psrow.tile([1, 512], F32, tag="row", name="ssk")
                    nc.tensor.matmul(
                        ssk[:], ones_col[:], sqs[(1, q4)][:],
                        start=True, stop=True,
                    )
                    lnk = rows.tile([1, 512], F32, tag="lnk", bufs=2)
                    nc.scalar.activation(
                        lnk[:], ssk[:], ActF.Ln,
                        scale=1.0 / D, bias=beps_col[0:1, :],
                    )
                    nc.scalar.activation(
                        rkrow[:, lsl], lnk[:], ActF.Exp, scale=-0.5,
                    )
                # rkd issues from the scalar queue (in-order after the exp
                # that produces rkrow); the DRAM->SBUF leg waits on the sync
                # queue so neither blocks the other engine's stream
                rkd = drp.tile([1, T2], F32, tag="rkd")
                nc.scalar.dma_start(rkd[:], rkrow[:])
                nc.sync.dma_start(
                    rk_cols[h][:, hf * TB2 : (hf + 1) * TB2],
                    rkd[0:1, :].rearrange("a (j p) -> a p j", p=128),
                )
                # q: rq = ssq^-1/2 (folds 1/sqrt(D); no eps -- pad heads
                # get nonzero Wq host-side), applied via ones-outer broadcast
                for q4 in range(Q42):
                    lsl = slice(q4 * 512, (q4 + 1) * 512)
                    ssq = psrow.tile([1, 512], F32, tag="row", name="ssq")
                    nc.tensor.matmul(
                        ssq[:], ones_col[:], sqs[(0, q4)][:],
                        start=True, stop=True,
                    )
                    lnq = rows.tile([1, 512], F32, tag="lnk", bufs=2, name="lnq")
                    nc.scalar.activation(lnq[:], ssq[:], ActF.Ln)
                    rwr = rows.tile([1, 512], BF16, tag="rwr", bufs=2)
                    nc.scalar.activation(rwr[:], lnq[:], ActF.Exp, scale=-0.5)
                    bq = psmm.tile([128, 512], F32, tag="mm", name="bq")
                    nc.tensor.matmul(
                        bq[:], ones_row[:], rwr[:], start=True, stop=True
                    )
                    nc.vector.tensor_mul(qtn[:, lsl], qtn[:, lsl], bq[:])

                pending = (lambda hf=hf, h=h, qtn=qtn, ytn=ytn:
                           emit_attention(hf, h, qtn, ytn))

            # ---- defer last head's attention + this pass's out projection
            # into the next pass (or the epilogue) so projection matmuls
            # hide the attention's exp-chain latency ----
            def make_post(hf=hf, ytn=ytn):
                def post():
                    for tb in range(TB2):
                        for ci, (co, cw) in enumerate(couts):
                            op = psacc.tile([128, cw], F32, tag="acc", name="op")
                            for hh in range(NHL):
                                nc.tensor.matmul(
                                    op[:],
                                    ytn[:, hh, tb * 128 : (tb + 1) * 128],
                                    wp[(hh, ci)][:],
                                    start=(hh == 0), stop=(hh == NHL - 1),
                                )
                            ot = oev.tile([128, cw], BF16, tag="ot")
                            nc.vector.tensor_copy(ot[:], op[:])
                            nc.sync.dma_start(
                                out[hf * T2 + tb * 128 : hf * T2 + (tb + 1) * 128,
                                    co : co + cw],
                                ot[:],
                            )
                return post

            post_attn = make_post()

        # epilogue: last head's attention of pass 1, then its out projection
        if pending is not None:
            norm_last = pending()
            norm_last()
            pending = None
        if post_attn is not None:
            post_attn()
            post_attn = None
    return nc


@functools.lru_cache(maxsize=4)
def _build(T_=T, C_=C, D_=D, NHL_=NHL, eps=EPS):
    import concourse.bacc as bacc
    import concourse.tile as tile
    from concourse import mybir

    nc = bacc.Bacc("TRN2", target_bir_lowering=False)
    _emit(nc, tile, mybir, T_, C_, D_, NHL_, eps)
    nc.compile()
    return nc


def _shard(x, cos, sin, Wq, Wk, Wv, Wproj):
    """Build the 8 per-core input maps."""
    import ml_dtypes

    BF = ml_dtypes.bfloat16
    cosT = np.ascontiguousarray(cos[0, 0].T.astype(np.float32))  # [64, T]
    sinT = np.ascontiguousarray(sin[0, 0].T.astype(np.float32))
    csd = np.concatenate([cosT, cosT], axis=0).astype(BF)  # [128, T]
    ssd = np.concatenate([sinT, -sinT], axis=0).astype(BF)

    def head_rows(W, heads, pad=0.0):
        rows = np.full((NHL * D, C), pad, np.float32)
        for i, h in enumerate(heads):
            rows[i * D : (i + 1) * D] = W[h * D : (h + 1) * D]
        return rows

    in_maps = []
    for b in range(B):
        xtb = np.ascontiguousarray(x[b].T).astype(BF)  # [C, T]
        for heads in GROUPS:
            wq = np.ascontiguousarray(head_rows(Wq, heads, pad=0.01).T).astype(BF)
            wk = np.ascontiguousarray(head_rows(Wk, heads).T).astype(BF)
            wv = np.ascontiguousarray(head_rows(Wv, heads).T).astype(BF)
            # Wproj columns for these heads, transposed: [HD, C]
            wpr = np.zeros((NHL * D, C), np.float32)
            for i, h in enumerate(heads):
                wpr[i * D : (i + 1) * D] = Wproj[:, h * D : (h + 1) * D].T
            in_maps.append(
                {"xt": xtb, "wqt": wq, "wkt": wk, "wvt": wv,
                 "wpt": wpr.astype(BF), "csd": csd, "ssd": ssd}
            )
    return in_maps


def _gather(results):
    y = np.zeros((B, T, C), np.float32)
    for b in range(B):
        for g in range(len(GROUPS)):
            y[b] += results[b * len(GROUPS) + g]["out"].astype(np.float32)
    return y


def _run(in_maps, trace=False):
    from concourse.bass_utils import run_bass_kernel_spmd

    nc = _build()
    return run_bass_kernel_spmd(
        nc, in_maps, core_ids=list(range(N_CORES)), trace=trace
    )


def kernel(x, cos, sin, Wq, Wk, Wv, Wproj):
    ins = _shard(
        np.asarray(x), np.asarray(cos), np.asarray(sin),
        np.asarray(Wq), np.asarray(Wk), np.asarray(Wv), np.asarray(Wproj),
    )
    res = _run(ins, trace=False)
    return _gather(res.results)


def run_traced(x, cos, sin, Wq, Wk, Wv, Wproj):
    ins = _shard(
        np.asarray(x), np.asarray(cos), np.asarray(sin),
        np.asarray(Wq), np.asarray(Wk), np.asarray(Wv), np.asarray(Wproj),
    )
    res = _run(ins, trace=True)
    return _gather(res.results), res


# revision 23
# speedup vs baseline: 1.1166x; 1.1166x over previous
"""Causal self-attention (RoPE + qk-RMS-norm) Trainium2 kernel.

Sharding: 8 cores = 2 batches x 4 head-groups (tensor-parallel over heads,
data-parallel over batch). Each core computes its head-group's attention and
a row-parallel partial of the output projection; the host sums the 4
per-group partials per batch (the all-reduce of row-parallel sharding).

All matmul operands are bf16 (PSUM accumulation stays fp32); norm scalars
and softmax denominators stay fp32. RoPE is computed without the tensor
engine: a SBUF->SBUF DMA swaps the two halves of the head dim, then
y = q*[cos;cos] + swap(q)*[sin;-sin] on the vector/gpsimd engines. The
RMS-norm sum-of-squares is taken from the pre-rope values (rotation
preserves norms), so the norm chain runs in parallel with the rope chain.
Attention matmuls (S.T, PV, colsum) and the exp are narrowed to the causal
span inside diagonal 512-query supertiles. Tokens are processed in two
causal passes; each pass's last-head attention is carried into the next
pass (or the epilogue) so projection matmuls hide its exp latency.
"""

import functools

import numpy as np

B, T, C, H, D = 2, 2048, 1280, 10, 128
EPS = 1e-5
NHL = 3  # head slots per core (padded)
N_CORES = 8
NHALF = 2  # causal passes over T
NWARM = 48
# per-batch head groups (4th group padded with zero heads)
GROUPS = [[0, 1, 2], [3, 4, 5], [6, 7, 8], [9]]


def _emit(nc, tile, mybir, T, C, D, NHL, eps):
    F32 = mybir.dt.float32
    F32R = mybir.dt.float32r
    BF16 = mybir.dt.bfloat16
    ActF = mybir.ActivationFunctionType
    CCH = C // 128  # contraction chunks
    TBN = T // 128  # 128-token blocks
    T2 = T // NHALF  # tokens per pass
    TB2 = T2 // 128
    Q42 = T2 // 512  # q supertiles per pass
    HD = NHL * D
    couts = []
    off = 0
    while off < C:
        w = min(512, C - off)
        couts.append((off, w))
        off += w

    xt = nc.dram_tensor("xt", [C, T], BF16, kind="ExternalInput")
    wqt = nc.dram_tensor("wqt", [C, HD], BF16, kind="ExternalInput")
    wkt = nc.dram_tensor("wkt", [C, HD], BF16, kind="ExternalInput")
    wvt = nc.dram_tensor("wvt", [C, HD], BF16, kind="ExternalInput")
    wpt = nc.dram_tensor("wpt", [HD, C], BF16, kind="ExternalInput")
    csd = nc.dram_tensor("csd", [D, T], BF16, kind="ExternalInput")  # [cos;cos]
    ssd = nc.dram_tensor("ssd", [D, T], BF16, kind="ExternalInput")  # [sin;-sin]
    out = nc.dram_tensor("out", [T, C], BF16, kind="ExternalOutput")

    from contextlib import ExitStack

    with ExitStack() as ctx:
        ctx.enter_context(nc.allow_low_precision(reason="bf16 matmul pipeline"))
        tc = ctx.enter_context(tile.TileContext(nc))
        pool = lambda n, b, **kw: ctx.enter_context(tc.tile_pool(name=n, bufs=b, **kw))
        drp = pool("dr", 2, space="DRAM")
        per = pool("persist", 1)
        wvp = pool("wv", 1)
        wqkp = pool("wqk", 1)
        wptp = pool("wpt", 1)
        xtp = pool("xt", 1)
        qtp = pool("qt", 2)
        qxp = pool("qx", 1)
        ytp = pool("yt", 2)
        tmp = pool("tmp", 4)
        sqp = pool("sqp", 1)
        ptp = pool("ptp", 4)
        rows = pool("rows", 3)
        oev = pool("oev", 2)
        psmm = pool("psmm", 2, space="PSUM")
        psacc = pool("psacc", 2, space="PSUM")
        psrow = pool("psrow", 2, space="PSUM")

        # ---- constants ----
        ones_f = per.tile([128, 128], F32, tag="onf")
        nc.vector.memset(ones_f[:], 1.0)
        ones_col = per.tile([128, 1], BF16, tag="onc")
        nc.scalar.copy(ones_col[:], ones_f[:, 0:1])
        ones_row = per.tile([1, 128], BF16, tag="onr")
        nc.scalar.copy(ones_row[:], ones_f[0:1, :])
        # broadcast row of 1/sqrt(D): folds the softmax scale into rq
        dsc_row = per.tile([1, 128], BF16, tag="dsc")
        nc.vector.memset(dsc_row[:], 1.0 / float(np.sqrt(D)))
        beps_col = per.tile([128, 1], F32, tag="bepsc")
        nc.vector.memset(beps_col[:], float(eps))
        # 0/1 mask: keep tq >= tk in [tk, tq] layout (upper incl diag)
        tri01 = per.tile([128, 128], BF16, tag="tri")
        nc.vector.memset(tri01[:], 1.0)
        nc.gpsimd.affine_select(
            out=tri01[:], in_=tri01[:],
            compare_op=mybir.AluOpType.is_ge,
            fill=0.0, base=0,
            pattern=[[1, 128]], channel_multiplier=-1,
        )

        # PE warm-up: dummy accumulating matmuls during the initial DMA ramp
        warm = nc.dram_tensor("warm", [1, 512], F32, kind="ExternalOutput")
        wrhs = per.tile([128, 512], BF16, tag="wrhs")
        for i in range(4):
            nc.scalar.copy(wrhs[:, i * 128 : (i + 1) * 128], ones_f[:])
        wps = psrow.tile([1, 512], F32, tag="row", name="warmps")
        for i in range(NWARM):
            nc.tensor.matmul(
                wps[:], ones_col[:], wrhs[:], start=(i == 0), stop=(i == NWARM - 1)
            )
        wsb = rows.tile([1, 512], F32, tag="rw", name="warmsb")
        nc.vector.tensor_copy(wsb[:], wps[:])
        nc.sync.dma_start(warm[:], wsb[:])

        # V for all heads/all tokens: [tk-part, tb, h, d]
        v_t = per.tile([128, TBN, NHL, D], BF16, tag="v")
        # K.T per head, all tokens
        ktr = [per.tile([128, T], BF16, tag=f"ktr{h}", name=f"ktr{h}")
               for h in range(NHL)]
        rk_cols = [per.tile([128, TBN], F32, tag=f"rkc{h}", name=f"rkc{h}")
                   for h in range(NHL)]

        # ---- all weights resident, loaded upfront on the sync queue ----
        # x.T for both passes: pass 0 on sync (needed first), pass 1 on gpsimd
        xc_all = []
        for hf in range(NHALF):
            xs = []
            for c in range(CCH):
                t = xtp.tile([128, T2], BF16, tag=f"x{c}p{hf}")
                eng = nc.sync if hf == 0 else nc.gpsimd
                eng.dma_start(
                    t[:], xt[c * 128 : (c + 1) * 128, hf * T2 : (hf + 1) * T2]
                )
                xs.append(t)
            xc_all.append(xs)
        wqall = []
        wkall = []
        for c in range(CCH):
            tq = wqkp.tile([128, HD], BF16, tag=f"wq{c}")
            nc.sync.dma_start(tq[:], wqt[c * 128 : (c + 1) * 128, :])
            wqall.append(tq)
            tk = wqkp.tile([128, HD], BF16, tag=f"wk{c}")
            nc.sync.dma_start(tk[:], wkt[c * 128 : (c + 1) * 128, :])
            wkall.append(tk)
        wv = []
        for c in range(CCH):
            t = wvp.tile([128, HD], BF16, tag=f"wv{c}")
            nc.sync.dma_start(t[:], wvt[c * 128 : (c + 1) * 128, :])
            wv.append(t)
        # cos/sin stacks for both passes
        cs_t = []
        ss_t = []
        for hf in range(NHALF):
            tc_ = qtp.tile([D, T2], BF16, tag=f"cs{hf}", bufs=1)
            ts_ = qtp.tile([D, T2], BF16, tag=f"ss{hf}", bufs=1)
            eng = nc.sync if hf == 0 else nc.gpsimd
            eng.dma_start(tc_[:], csd[:, hf * T2 : (hf + 1) * T2])
            eng.dma_start(ts_[:], ssd[:, hf * T2 : (hf + 1) * T2])
            cs_t.append(tc_)
            ss_t.append(ts_)
        # output-projection weights (resident)
        wp = {}
        for hh in range(NHL):
            for ci, (co, cw) in enumerate(couts):
                t = wptp.tile([128, cw], BF16, tag=f"wp{hh}_{ci}")
                nc.sync.dma_start(
                    t[:], wpt[hh * 128 : (hh + 1) * 128, co : co + cw]
                )
                wp[(hh, ci)] = t

        def emit_attention(hf, h, qtn, ytn):
            """Attention for head h over this pass's q supertiles.
            kb-outer; st/exp run LA kb steps ahead of PV/colsum. Matmuls and
            the exp are narrowed to the causal span inside diagonal
            supertiles."""
            gq4s = [hf * Q42 + q4 for q4 in range(Q42)]
            yts = [psacc.tile([128, 512], F32, tag="acc", name=f"yt{q4}")
                   for q4 in range(Q42)]
            csums = [psrow.tile([1, 512], F32, tag="row", name=f"cs{q4}")
                     for q4 in range(Q42)]
            kbmax = 4 * (gq4s[-1] + 1)
            LA = 3  # st/exp run this many kb steps ahead of PV/colsum
            pts = {}  # kb -> pt tile awaiting PV/colsum

            def span(q4, kb):
                # valid columns of supertile q4 at key block kb (None if none)
                j = kb - 4 * gq4s[q4]
                if j > 3:
                    return None
                return max(j, 0) * 128

            for kb in range(kbmax + LA):
                if kb < kbmax:
                    active = [q4 for q4 in range(Q42) if kb <= 4 * gq4s[q4] + 3]
                    st = psmm.tile([128, Q42 * 512], F32, tag="mm", name="st")
                    for q4 in active:
                        a0 = q4 * 512 + span(q4, kb)
                        hi4 = (q4 + 1) * 512
                        nc.tensor.matmul(
                            st[:, a0:hi4],
                            ktr[h][:, kb * 128 : (kb + 1) * 128],
                            qtn[:, a0:hi4],
                            start=True, stop=True,
                        )
                    pt = ptp.tile([128, Q42 * 512], BF16, tag="pt")
                    lo = active[0] * 512 + span(active[0], kb)
                    hi = (active[-1] + 1) * 512
                    nc.scalar.activation(
                        pt[:, lo:hi], st[:, lo:hi], ActF.Exp,
                        scale=rk_cols[h][:, kb : kb + 1],
                    )
                    for q4 in active:
                        j = kb - 4 * gq4s[q4]
                        if 0 <= j <= 3:
                            dg = slice(q4 * 512 + j * 128, q4 * 512 + (j + 1) * 128)
                            nc.vector.tensor_mul(pt[:, dg], pt[:, dg], tri01[:])
                    pts[kb] = pt
                if kb >= LA:
                    pkb = kb - LA
                    pt = pts.pop(pkb)
                    for q4 in range(Q42):
                        gq4 = gq4s[q4]
                        last_kb = 4 * gq4 + 3
                        if pkb > last_kb:
                            continue
                        a0 = span(q4, pkb)
                        nc.tensor.matmul(
                            yts[q4][:, a0:512],
                            v_t[:, pkb, h, :],
                            pt[:, q4 * 512 + a0 : (q4 + 1) * 512],
                            start=(pkb == 0), stop=(pkb == last_kb),
                            skip_group_check=True,
                        )
                        nc.tensor.matmul(
                            csums[q4][:, a0:512],
                            ones_col[:],
                            pt[:, q4 * 512 + a0 : (q4 + 1) * 512],
                            start=(pkb == 0), stop=(pkb == last_kb),
                            skip_group_check=True,
                        )
            # 1/colsum rows: recip straight from PSUM, then a rounding copy
            # into F32R (walrus requires fp32r matmul inputs to be rounded)
            rrrs = []
            for q4 in range(Q42):
                rr = rows.tile([1, 512], F32, tag="rw", name="rr")
                nc.vector.reciprocal_approx_fast(rr[:], csums[q4][:])
                rrr = rows.tile([1, 512], BF16, tag="rr", name="rrr")
                nc.vector.tensor_copy(rrr[:], rr[:])
                rrrs.append(rrr)

            def normalize(h=h, ytn=ytn, yts=yts, rrrs=rrrs):
                for q4 in range(Q42):
                    lsl = slice(q4 * 512, (q4 + 1) * 512)
                    bc = psmm.tile([128, 512], F32, tag="mm", name="bc")
                    nc.tensor.matmul(
                        bc[:], ones_row[:], rrrs[q4][:], start=True, stop=True
                    )
                    # DVE can read only one PSUM operand: evict bc via ACT
                    bcb = tmp.tile([128, 512], F32, tag="bcb")
                    nc.scalar.copy(bcb[:], bc[:])
                    nc.vector.tensor_mul(ytn[:, h, lsl], yts[q4][:], bcb[:])

            return normalize

        pending = None  # deferred attention emitter for the previous head
        post_attn = None  # deferred out-projection for the previous pass

        for hf in range(NHALF):
            toff = hf * T2
            xc = xc_all[hf]
            cst = cs_t[hf]
            sst = ss_t[hf]

            def emit_vproj(hf=hf, xc=xc):
                # V projection for this pass, all heads batched. Emitted
                # after head 0's Q/K projections so its PE matmuls cover
                # head 0's rope/norm chain latency.
                for tb in range(TB2):
                    gtb = hf * TB2 + tb
                    vp = psmm.tile([128, HD], F32, tag="mm", name="vp")
                    for c in range(CCH):
                        nc.tensor.matmul(
                            vp[:],
                            xc[c][:, tb * 128 : (tb + 1) * 128],
                            wv[c][:],
                            start=(c == 0), stop=(c == CCH - 1),
                        )
                    nc.vector.tensor_copy(v_t[:, gtb, :, :], vp[:])

            # Y.T for this pass (all heads)
            ytn = ytp.tile([128, NHL, T2], BF16, tag="ytn")

            for h in range(NHL):
                # ---- Q/K projections into PSUM; evict + swap + squares ----
                qunits = {}  # (isq, q4) -> (qx, qsw, sq)
                for isq, wt in enumerate((wqall, wkall)):
                    qps = psmm.tile([128, Q42 * 512], F32, tag="mm", name="qps")
                    for c in range(CCH):
                        for q4 in range(Q42):
                            nc.tensor.matmul(
                                qps[:, q4 * 512 : (q4 + 1) * 512],
                                wt[c][:, h * D : (h + 1) * D],
                                xc[c][:, q4 * 512 : (q4 + 1) * 512],
                                start=(c == 0), stop=(c == CCH - 1),
                            )
                    for q4 in range(Q42):
                        qx = qxp.tile([128, 512], BF16, tag=f"qx{isq}{q4}")
                        nc.vector.tensor_copy(qx[:], qps[:, q4 * 512 : (q4 + 1) * 512])
                        qsw = qxp.tile([128, 512], BF16, tag=f"qw{isq}{q4}")
                        nc.sync.dma_start(qsw[0:64, :], qx[64:128, :])
                        nc.sync.dma_start(qsw[64:128, :], qx[0:64, :])
                        qunits[(isq, q4)] = (qx, qsw)

                # ---- previous head's attention (dense PE block) ----
                if pending is not None:
                    norm_prev = pending()
                    pending = None
                else:
                    norm_prev = None

                qtn = qtp.tile([128, T2], BF16, tag="qtn")

                if norm_prev is not None:
                    norm_prev()
                if post_attn is not None:
                    post_attn()
                    post_attn = None
                if h == 0:
                    emit_vproj()

                # ---- rope combine (no PE): dst = qx*CS + swap(qx)*SS ----
                # (cos/sin here are arbitrary tensors, so rope does NOT
                # preserve norms -- squares must come from post-rope values)
                combs = {}
                for isq in range(2):
                    for q4 in range(Q42):
                        qx, qsw = qunits[(isq, q4)]
                        lsl4 = slice(q4 * 512, (q4 + 1) * 512)
                        qc = tmp.tile([128, 512], BF16, tag="t1")
                        nc.vector.tensor_mul(qc[:], qx[:], cst[:, lsl4])
                        t2 = tmp.tile([128, 512], BF16, tag="t2")
                        nc.gpsimd.tensor_mul(t2[:], qsw[:], sst[:, lsl4])
                        combs[(isq, q4)] = (qc, t2)
                sqs = {}
                for isq, (dst, doff) in enumerate(((qtn, 0), (ktr[h], toff))):
                    for q4 in range(Q42):
                        qc, t2 = combs[(isq, q4)]
                        dsl = slice(doff + q4 * 512, doff + (q4 + 1) * 512)
                        nc.vector.tensor_add(dst[:, dsl], qc[:], t2[:])
                        sq = sqp.tile([128, 512], BF16, tag=f"sq{isq}{q4}")
                        nc.vector.tensor_mul(sq[:], dst[:, dsl], dst[:, dsl])
                        sqs[(isq, q4)] = sq

                # ---- norms from post-rope squares ----
                # All four sum-of-squares rows land in ONE psum bank at
                # partitions 0/32/64/96 (legal matmul tile positions), then
                # rsqrt runs entirely on DVE (quake bit-trick + one Newton
                # step, ~0.1% rel err). The scalar engine keeps only Exp and
                # Copy -- one activation table, zero reloads.
                nsq = psrow.tile([128, 512], F32, tag="row", name="nsq")
                for i, (isq, q4) in enumerate(
                    ((0, 0), (0, 1), (1, 0), (1, 1))[: 2 * Q42]
                ):
                    nc.tensor.matmul(
                        nsq[32 * i : 32 * i + 1, :], ones_col[:],
                        sqs[(isq, q4)][:], start=True, stop=True,
                        skip_group_check=True, tile_position=(0, 32 * i),
                    )
                I32 = mybir.dt.int32
                # v = x/D + eps; rows 0/32 are ssq (q), 64/96 are ssk (k).
                # (q picks up a sqrt(D) factor, cancelled by dsc_row in bq.)
                nv = tmp.tile([128, 512], F32, tag="nv")
                nc.vector.tensor_scalar(
                    nv[:], nsq[:], 1.0 / D, float(eps),
                    mybir.AluOpType.mult, mybir.AluOpType.add,
                )
                ny = tmp.tile([128, 512], F32, tag="ny")
                nc.vector.tensor_scalar(
                    ny[:].bitcast(I32), nv[:].bitcast(I32), 1, None,
                    mybir.AluOpType.logical_shift_right,
                )
                nc.vector.tensor_scalar(
                    ny[:].bitcast(I32), ny[:].bitcast(I32), -1, 0x5F3759DF,
                    mybir.AluOpType.mult, mybir.AluOpType.add,
                )
                nw = tmp.tile([128, 512], F32, tag="nw")
                nc.vector.tensor_mul(nw[:], nv[:], ny[:])
                nc.vector.tensor_mul(nw[:], nw[:], ny[:])
                nc.vector.tensor_scalar(
                    nw[:], nw[:], -0.5, 1.5,
                    mybir.AluOpType.mult, mybir.AluOpType.add,
                )
                nc.vector.tensor_mul(ny[:], ny[:], nw[:])
                # k rows -> rk_cols via the DRAM bounce (strided transpose);
                # the SBUF->DRAM leg on the scalar queue, DRAM->SBUF on sync
                rkd = drp.tile([1, T2], F32, tag="rkd")
                nc.scalar.dma_start(rkd[0:1, 0:512], ny[64:65, :])
                nc.scalar.dma_start(rkd[0:1, 512:1024], ny[96:97, :])
                nc.sync.dma_start(
                    rk_cols[h][:, hf * TB2 : (hf + 1) * TB2],
                    rkd[0:1, :].rearrange("a (j p) -> a p j", p=128),
                )
                # q rows: rwr = rq*sqrt(D)... broadcast by dsc_row=1/sqrt(D)
                for q4 in range(Q42):
                    lsl = slice(q4 * 512, (q4 + 1) * 512)
                    rwr = rows.tile([1, 512], BF16, tag="rwr", bufs=2)
                    nc.vector.tensor_copy(rwr[:], ny[32 * q4 : 32 * q4 + 1, :])
                    bq = psmm.tile([128, 512], F32, tag="mm", name="bq")
                    nc.tensor.matmul(
                        bq[:], dsc_row[:], rwr[:], start=True, stop=True
                    )
                    nc.vector.tensor_mul(qtn[:, lsl], qtn[:, lsl], bq[:])

                pending = (lambda hf=hf, h=h, qtn=qtn, ytn=ytn:
                           emit_attention(hf, h, qtn, ytn))

            # ---- defer last head's attention + this pass's out projection
            # into the next pass (or the epilogue) so projection matmuls
            # hide the attention's exp-chain latency ----
            def make_post(hf=hf, ytn=ytn):
                def post():
                    for tb in range(TB2):
                        for ci, (co, cw) in enumerate(couts):
                            op = psacc.tile([128, cw], F32, tag="acc", name="op")
                            for hh in range(NHL):
                                nc.tensor.matmul(
                                    op[:],
                                    ytn[:, hh, tb * 128 : (tb + 1) * 128],
                                    wp[(hh, ci)][:],
                                    start=(hh == 0), stop=(hh == NHL - 1),
                                )
                            ot = oev.tile([128, cw], BF16, tag="ot")
                            nc.vector.tensor_copy(ot[:], op[:])
                            nc.sync.dma_start(
                                out[hf * T2 + tb * 128 : hf * T2 + (tb + 1) * 128,
                                    co : co + cw],
                                ot[:],
                            )
                return post

            post_attn = make_post()

        # epilogue: last head's attention of pass 1, then its out projection
        if pending is not None:
            norm_last = pending()
            norm_last()
            pending = None
        if post_attn is not None:
            post_attn()
            post_attn = None
    return nc


@functools.lru_cache(maxsize=4)
def _build(T_=T, C_=C, D_=D, NHL_=NHL, eps=EPS):
    import concourse.bacc as bacc
    import concourse.tile as tile
    from concourse import mybir

    nc = bacc.Bacc("TRN2", target_bir_lowering=False)
    _emit(nc, tile, mybir, T_, C_, D_, NHL_, eps)
    nc.compile()
    return nc


def _shard(x, cos, sin, Wq, Wk, Wv, Wproj):
    """Build the 8 per-core input maps."""
    import ml_dtypes

    BF = ml_dtypes.bfloat16
    cosT = np.ascontiguousarray(cos[0, 0].T.astype(np.float32))  # [64, T]
    sinT = np.ascontiguousarray(sin[0, 0].T.astype(np.float32))
    csd = np.concatenate([cosT, cosT], axis=0).astype(BF)  # [128, T]
    ssd = np.concatenate([sinT, -sinT], axis=0).astype(BF)

    def head_rows(W, heads, pad=0.0):
        rows = np.full((NHL * D, C), pad, np.float32)
        for i, h in enumerate(heads):
            rows[i * D : (i + 1) * D] = W[h * D : (h + 1) * D]
        return rows

    in_maps = []
    for b in range(B):
        xtb = np.ascontiguousarray(x[b].T).astype(BF)  # [C, T]
        for heads in GROUPS:
            wq = np.ascontiguousarray(head_rows(Wq, heads, pad=0.01).T).astype(BF)
            wk = np.ascontiguousarray(head_rows(Wk, heads).T).astype(BF)
            wv = np.ascontiguousarray(head_rows(Wv, heads).T).astype(BF)
            # Wproj columns for these heads, transposed: [HD, C]
            wpr = np.zeros((NHL * D, C), np.float32)
            for i, h in enumerate(heads):
                wpr[i * D : (i + 1) * D] = Wproj[:, h * D : (h + 1) * D].T
            in_maps.append(
                {"xt": xtb, "wqt": wq, "wkt": wk, "wvt": wv,
                 "wpt": wpr.astype(BF), "csd": csd, "ssd": ssd}
            )
    return in_maps


def _gather(results):
    y = np.zeros((B, T, C), np.float32)
    for b in range(B):
        for g in range(len(GROUPS)):
            y[b] += results[b * len(GROUPS) + g]["out"].astype(np.float32)
    return y


def _run(in_maps, trace=False):
    from concourse.bass_utils import run_bass_kernel_spmd

    nc = _build()
    return run_bass_kernel_spmd(
        nc, in_maps, core_ids=list(range(N_CORES)), trace=trace
    )


def kernel(x, cos, sin, Wq, Wk, Wv, Wproj):
    ins = _shard(
        np.asarray(x), np.asarray(cos), np.asarray(sin),
        np.asarray(Wq), np.asarray(Wk), np.asarray(Wv), np.asarray(Wproj),
    )
    res = _run(ins, trace=False)
    return _gather(res.results)


def run_traced(x, cos, sin, Wq, Wk, Wv, Wproj):
    ins = _shard(
        np.asarray(x), np.asarray(cos), np.asarray(sin),
        np.asarray(Wq), np.asarray(Wk), np.asarray(Wv), np.asarray(Wproj),
    )
    res = _run(ins, trace=True)
    return _gather(res.results), res
